# revision 6
# baseline (speedup 1.0000x reference)
"""Trainium2 Bass kernel for nn_Memory_63599875719529 (retrieval_knn).

Pipeline: cosine-sim (512x256) -> top-16 per row -> clamp/renorm weights ->
dense (512,256)@(256,131072) GEMM against the memory bank.

Sharding: output columns (the flattened 64*2048 prompt dims) are split
across the 8 cores (16384 cols each). Each core reads only its 1/8 slice of
the 134MB memory bank and writes its 1/8 slice of the 268MB output — no
collectives. The cheap sim/top-k/weights part is replicated on every core.

Numerics:
  - sim matmul in fp32 (PE 4 cyc/row): the 16th/17th neighbour gap can be
    as small as 1.6e-6, so selection must be fp32-exact.
  - top-16 via DVE max8 + match_replace (2 rounds), exact fp32 values.
  - big GEMM in float32r (TF32-like, 1 cyc/row, rel err ~1.5e-4); inputs
    are rounded to fp32r by the producing copy as the HW requires.
"""

import numpy as np

B = 512          # batch (features rows)
D = 512          # feature dim
M = 256          # memory size
PQ = 64 * 2048   # flattened prompt shape
N_CORES = 8
NSH = PQ // N_CORES  # 16384 output cols per core
P = 128

NT_CHUNK = 2048  # columns loaded/computed per GEMM step
N_CHUNKS = NSH // NT_CHUNK

_CACHED_NC = None


def _build_nc():
    import concourse.bass as bass  # noqa: F401  (registers types)
    import concourse.tile as tile
    from concourse import bacc, mybir
    from concourse.masks import make_identity

    f32 = mybir.dt.float32
    f32r = mybir.dt.float32r

    nc = bacc.Bacc("TRN2", target_bir_lowering=False, debug=False)
    features = nc.dram_tensor("features", [B, D], f32, kind="ExternalInput")
    keys = nc.dram_tensor("keys", [M, D], f32, kind="ExternalInput")
    mem = nc.dram_tensor("mem", [M, NSH], f32, kind="ExternalInput")
    out = nc.dram_tensor("out", [B, NSH], f32, kind="ExternalOutput")

    fap = features.ap()
    kap = keys.ap()
    map_ = mem.ap()
    oap = out.ap()

    FB = B // P   # 4 feature row-blocks
    KB = M // P   # 2 key row-blocks
    DC = D // P   # 4 contraction chunks

    with tile.TileContext(nc) as tc:
        with (
            tc.tile_pool(name="persist", bufs=1) as persist,
            tc.tile_pool(name="scratch", bufs=2) as scratch,
            tc.tile_pool(name="mem_f", bufs=3) as mem_f_pool,
            tc.tile_pool(name="mem_r", bufs=3) as mem_r_pool,
            tc.tile_pool(name="outp", bufs=6) as out_pool,
            tc.tile_pool(name="ps1", bufs=2, space="PSUM") as ps1,
            tc.tile_pool(name="psg", bufs=4, space="PSUM") as psg,
        ):
            ident = persist.tile([P, P], f32, tag="ident", name="ident")
            make_identity(nc, ident[:])

            # Prefetch + convert the first memory chunks before phase 1 so
            # the GEMM can start the moment the weights are ready. In-DMAs
            # go through GpSimd (SWDGE) so they never queue behind the
            # out-DMA triggers on the Sync engine.
            def load_chunk(nt):
                pair = []
                for kb in range(KB):
                    mf = mem_f_pool.tile(
                        [P, NT_CHUNK], f32, tag=f"memf{kb}", name=f"memf{kb}_{nt}"
                    )
                    nc.gpsimd.dma_start(
                        mf[:],
                        map_[kb * P : (kb + 1) * P,
                             nt * NT_CHUNK : (nt + 1) * NT_CHUNK],
                    )
                    mr = mem_r_pool.tile(
                        [P, NT_CHUNK], f32r, tag=f"memr{kb}", name=f"memr{kb}_{nt}"
                    )
                    nc.vector.tensor_copy(mr[:], mf[:])
                    pair.append(mr)
                return pair

            preloaded = {nt: load_chunk(nt) for nt in range(min(2, N_CHUNKS))}

            # ---- Phase 1: weights W (replicated on every core) ----
            # Load keys, normalize rows (features norm cancels out of the
            # final weights, so features are used unnormalized).
            kn = []
            for kb in range(KB):
                k_nat = persist.tile([P, D], f32, tag=f"k_nat{kb}", name=f"k_nat{kb}")
                nc.sync.dma_start(k_nat[:], kap[kb * P : (kb + 1) * P, :])
                sq = scratch.tile([P, D], f32, tag="sq", name="sq")
                ss = persist.tile([P, 1], f32, tag=f"ss{kb}", name=f"ss{kb}")
                nc.scalar.activation(
                    sq[:], k_nat[:], mybir.ActivationFunctionType.Square,
                    accum_out=ss[:],
                )
                nrm = persist.tile([P, 1], f32, tag=f"nrm{kb}", name=f"nrm{kb}")
                nc.scalar.sqrt(nrm[:], ss[:])
                nc.vector.tensor_scalar_max(nrm[:], nrm[:], 1e-8)
                rinv = persist.tile([P, 1], f32, tag=f"rinv{kb}", name=f"rinv{kb}")
                nc.vector.reciprocal(rinv[:], nrm[:])
                k_n = persist.tile([P, D], f32, tag=f"k_n{kb}", name=f"k_n{kb}")
                nc.vector.tensor_scalar_mul(k_n[:], k_nat[:], rinv[:])
                kn.append(k_n)

            # Load features and transpose both (contraction dim must be on
            # partitions for the PE).
            f_nat = []
            for fb in range(FB):
                t = persist.tile([P, D], f32, tag=f"f_nat{fb}", name=f"f_nat{fb}")
                nc.sync.dma_start(t[:], fap[fb * P : (fb + 1) * P, :])
                f_nat.append(t)

            ft = [persist.tile([P, B], f32, tag=f"ft{dc}", name=f"ft{dc}") for dc in range(DC)]
            for dc in range(DC):
                for fb in range(FB):
                    pt = ps1.tile([P, P], f32, tag="ps_tr", name="ps_tr")
                    nc.tensor.transpose(
                        pt[:], f_nat[fb][:, dc * P : (dc + 1) * P], ident[:]
                    )
                    nc.scalar.copy(ft[dc][:, fb * P : (fb + 1) * P], pt[:])

            knt = [persist.tile([P, M], f32, tag=f"knt{dc}", name=f"knt{dc}") for dc in range(DC)]
            for dc in range(DC):
                for kb in range(KB):
                    pt = ps1.tile([P, P], f32, tag="ps_tr", name="ps_tr")
                    nc.tensor.transpose(
                        pt[:], kn[kb][:, dc * P : (dc + 1) * P], ident[:]
                    )
                    nc.scalar.copy(knt[dc][:, kb * P : (kb + 1) * P], pt[:])

            # sim = F @ Kn^T, per 128-row block, fp32 accumulation in PSUM.
            w_sb = [persist.tile([P, M], f32, tag=f"w{fb}", name=f"w{fb}") for fb in range(FB)]
            for fb in range(FB):
                ps_sim = ps1.tile([P, M], f32, tag="ps_sim", name="ps_sim")
                for dc in range(DC):
                    nc.tensor.matmul(
                        ps_sim[:],
                        ft[dc][:, fb * P : (fb + 1) * P],
                        knt[dc][:],
                        start=(dc == 0),
                        stop=(dc == DC - 1),
                    )
                sim = persist.tile([P, M], f32, tag=f"sim{fb}", name=f"sim{fb}")
                nc.vector.tensor_copy(sim[:], ps_sim[:])

                # top-16: two rounds of (top-8, zap-to-0). All top-16 sims
                # are > 0 for this distribution, so 0 never wins a max.
                t = scratch.tile([P, M], f32, tag="tk_t", name="tk_t")
                m8a = scratch.tile([P, 8], f32, tag="tk_m8a", name="tk_m8a")
                m8b = scratch.tile([P, 8], f32, tag="tk_m8b", name="tk_m8b")
                nc.vector.max(out=m8a[:], in_=sim[:])
                nc.vector.match_replace(
                    out=t[:], in_to_replace=m8a[:], in_values=sim[:], imm_value=0.0
                )
                nc.vector.max(out=m8b[:], in_=t[:])
                nc.vector.match_replace(
                    out=t[:], in_to_replace=m8b[:], in_values=t[:], imm_value=0.0
                )
                # v = sim - t: top-16 keep value, rest -> 0
                v = scratch.tile([P, M], f32, tag="tk_v", name="tk_v")
                nc.vector.tensor_sub(out=v[:], in0=sim[:], in1=t[:])
                # clamp negatives + row sum in one ACT op
                v2 = scratch.tile([P, M], f32, tag="tk_v2", name="tk_v2")
                rowsum = scratch.tile([P, 1], f32, tag="tk_rs", name="tk_rs")
                nc.scalar.activation(
                    v2[:], v[:], mybir.ActivationFunctionType.Relu,
                    accum_out=rowsum[:],
                )
                rs_inv = scratch.tile([P, 1], f32, tag="tk_rsi", name="tk_rsi")
                nc.vector.reciprocal(rs_inv[:], rowsum[:])
                nc.vector.tensor_scalar_mul(w_sb[fb][:], v2[:], rs_inv[:])

            # Transpose W -> WT (fp32r, rounded by the copy out of PSUM).
            wt = [persist.tile([P, B], f32r, tag=f"wt{kb}", name=f"wt{kb}") for kb in range(KB)]
            for kb in range(KB):
                for fb in range(FB):
                    pt = ps1.tile([P, P], f32, tag="ps_tr", name="ps_tr")
                    nc.tensor.transpose(
                        pt[:], w_sb[fb][:, kb * P : (kb + 1) * P], ident[:]
                    )
                    nc.vector.tensor_copy(wt[kb][:, fb * P : (fb + 1) * P], pt[:])

            # ---- Phase 2: out = W @ mem, fp32r, streamed over columns ----
            SUBS = NT_CHUNK // 512
            for nt in range(N_CHUNKS):
                mem_r = preloaded.pop(nt) if nt in preloaded else load_chunk(nt)
                for fb in range(FB):
                    ot = out_pool.tile([P, NT_CHUNK], f32, tag="ot", name="ot")
                    for sub in range(SUBS):
                        ps = psg.tile([P, 512], f32, tag="ps_gemm", name="ps_gemm")
                        for kb in range(KB):
                            nc.tensor.matmul(
                                ps[:],
                                wt[kb][:, fb * P : (fb + 1) * P],
                                mem_r[kb][:, sub * 512 : (sub + 1) * 512],
                                start=(kb == 0),
                                stop=(kb == KB - 1),
                            )
                        dst = ot[:, sub * 512 : (sub + 1) * 512]
                        if sub % 2 == 0:
                            nc.vector.tensor_copy(dst, ps[:])
                        else:
                            nc.scalar.copy(dst, ps[:])
                    nc.sync.dma_start(
                        oap[fb * P : (fb + 1) * P,
                            nt * NT_CHUNK : (nt + 1) * NT_CHUNK],
                        ot[:],
                    )

    nc.finalize()
    return nc


def _get_nc():
    global _CACHED_NC
    if _CACHED_NC is None:
        _CACHED_NC = _build_nc()
    return _CACHED_NC


def kernel(features: np.ndarray, keys: np.ndarray, memory: np.ndarray) -> np.ndarray:
    from concourse.bass_utils import run_bass_kernel_spmd

    features = np.ascontiguousarray(np.asarray(features, dtype=np.float32))
    keys = np.ascontiguousarray(np.asarray(keys, dtype=np.float32))
    mem2d = np.asarray(memory, dtype=np.float32).reshape(M, PQ)

    in_maps = []
    for c in range(N_CORES):
        shard = np.ascontiguousarray(mem2d[:, c * NSH : (c + 1) * NSH])
        in_maps.append({"features": features, "keys": keys, "mem": shard})

    nc = _get_nc()
    last_err = None
    for _attempt in range(2):
        try:
            res = run_bass_kernel_spmd(nc, in_maps, core_ids=list(range(N_CORES)))
            break
        except Exception as e:  # transient NRT device errors: retry once
            last_err = e
    else:
        raise last_err

    out = np.concatenate([r["out"] for r in res.results], axis=1)
    return out.reshape(B, 64, 2048)


# revision 8
# speedup vs baseline: 1.1208x; 1.1208x over previous
"""Trainium2 Bass kernel for nn_Memory_63599875719529 (retrieval_knn).

Pipeline: cosine-sim (512x256) -> top-16 per row -> clamp/renorm weights ->
dense (512,256)@(256,131072) GEMM against the memory bank.

Sharding: output columns (the flattened 64*2048 prompt dims) are split
across the 8 cores (16384 cols each). Each core reads only its 1/8 slice of
the 134MB memory bank and writes its 1/8 slice of the 268MB output — no
collectives. The cheap sim/top-k/weights part is replicated on every core.

Numerics:
  - sim matmul in fp32 (PE 4 cyc/row): the 16th/17th neighbour gap can be
    as small as 1.6e-6, so selection must be fp32-exact.
  - top-16 via DVE max8 + match_replace (2 rounds), exact fp32 values.
  - big GEMM in float32r (TF32-like, 1 cyc/row, rel err ~1.5e-4); inputs
    are rounded to fp32r by the producing copy as the HW requires.

Scheduling notes (from profiling):
  - memory-bank in-DMAs go through GpSimd (SWDGE) so they never queue
    behind out-DMA triggers on the Sync engine's FIFO.
  - first chunks are prefetched before phase 1 so the GEMM starts the
    moment the weights are ready; PE must stay busy or the HAM clock
    gate re-throttles it to 1.2GHz.
  - transposes are grouped 4-per-PSUM-bank so one copy moves 512 cols.
"""

import numpy as np

B = 512          # batch (features rows)
D = 512          # feature dim
M = 256          # memory size
PQ = 64 * 2048   # flattened prompt shape
N_CORES = 8
NSH = PQ // N_CORES  # 16384 output cols per core
P = 128

NT_CHUNK = 1024  # columns loaded/computed per GEMM step
N_CHUNKS = NSH // NT_CHUNK
PRELOAD = 3      # chunks prefetched before phase 1

_CACHED_NC = None


def _build_nc():
    import concourse.bass as bass  # noqa: F401  (registers types)
    import concourse.tile as tile
    from concourse import bacc, mybir
    from concourse.masks import make_identity

    f32 = mybir.dt.float32
    f32r = mybir.dt.float32r
    AFT = mybir.ActivationFunctionType

    nc = bacc.Bacc("TRN2", target_bir_lowering=False, debug=False)
    features = nc.dram_tensor("features", [B, D], f32, kind="ExternalInput")
    keys = nc.dram_tensor("keys", [M, D], f32, kind="ExternalInput")
    mem = nc.dram_tensor("mem", [M, NSH], f32, kind="ExternalInput")
    out = nc.dram_tensor("out", [B, NSH], f32, kind="ExternalOutput")

    fap = features.ap()
    kap = keys.ap()
    map_ = mem.ap()
    oap = out.ap()

    FB = B // P   # 4 feature row-blocks
    KB = M // P   # 2 key row-blocks
    DC = D // P   # 4 contraction chunks
    SUBS = NT_CHUNK // 512

    with tile.TileContext(nc) as tc:
        with (
            tc.tile_pool(name="persist", bufs=1) as persist,
            tc.tile_pool(name="scratch", bufs=2) as scratch,
            tc.tile_pool(name="mem_f", bufs=4) as mem_f_pool,
            tc.tile_pool(name="mem_r", bufs=4) as mem_r_pool,
            tc.tile_pool(name="outp", bufs=8) as out_pool,
            tc.tile_pool(name="ps1", bufs=2, space="PSUM") as ps1,
            tc.tile_pool(name="psg", bufs=4, space="PSUM") as psg,
        ):
            # ---- Prefetch first memory chunks (GpSimd/SWDGE queue) ----
            def load_chunk(nt):
                pair = []
                for kb in range(KB):
                    mf = mem_f_pool.tile(
                        [P, NT_CHUNK], f32, tag=f"memf{kb}", name=f"memf{kb}_{nt}"
                    )
                    nc.gpsimd.dma_start(
                        mf[:],
                        map_[kb * P : (kb + 1) * P,
                             nt * NT_CHUNK : (nt + 1) * NT_CHUNK],
                    )
                    mr = mem_r_pool.tile(
                        [P, NT_CHUNK], f32r, tag=f"memr{kb}", name=f"memr{kb}_{nt}"
                    )
                    nc.vector.tensor_copy(mr[:], mf[:])
                    pair.append(mr)
                return pair

            # inputs for phase 1 first (tiny, on Sync queue)
            k_nat = []
            for kb in range(KB):
                t = persist.tile([P, D], f32, tag=f"k_nat{kb}", name=f"k_nat{kb}")
                nc.sync.dma_start(t[:], kap[kb * P : (kb + 1) * P, :])
                k_nat.append(t)
            f_nat = []
            for fb in range(FB):
                t = persist.tile([P, D], f32, tag=f"f_nat{fb}", name=f"f_nat{fb}")
                nc.sync.dma_start(t[:], fap[fb * P : (fb + 1) * P, :])
                f_nat.append(t)

            preloaded = {nt: load_chunk(nt) for nt in range(min(PRELOAD, N_CHUNKS))}

            ident = persist.tile([P, P], f32, tag="ident", name="ident")
            make_identity(nc, ident[:])

            # ---- Phase 1: weights W (replicated on every core) ----
            # Normalize key rows; feature norms cancel out of the weights.
            kn = []
            for kb in range(KB):
                sq = scratch.tile([P, D], f32, tag="sq", name="sq")
                ss = persist.tile([P, 1], f32, tag=f"ss{kb}", name=f"ss{kb}")
                nc.scalar.activation(sq[:], k_nat[kb][:], AFT.Square, accum_out=ss[:])
                nrm = persist.tile([P, 1], f32, tag=f"nrm{kb}", name=f"nrm{kb}")
                nc.scalar.sqrt(nrm[:], ss[:])
                nc.vector.tensor_scalar_max(nrm[:], nrm[:], 1e-8)
                rinv = persist.tile([P, 1], f32, tag=f"rinv{kb}", name=f"rinv{kb}")
                nc.vector.reciprocal(rinv[:], nrm[:])
                k_n = persist.tile([P, D], f32, tag=f"k_n{kb}", name=f"k_n{kb}")
                nc.vector.tensor_scalar_mul(k_n[:], k_nat[kb][:], rinv[:])
                kn.append(k_n)

            # Transpose F and Kn: 4 PE transposes into one PSUM bank, then a
            # single wide copy out. dc-interleaved; copies alternate ACT/DVE.
            ft = [
                persist.tile([P, B], f32, tag=f"ft{dc}", name=f"ft{dc}")
                for dc in range(DC)
            ]
            knt = [
                persist.tile([P, M], f32, tag=f"knt{dc}", name=f"knt{dc}")
                for dc in range(DC)
            ]
            for dc in range(DC):
                ptf = ps1.tile([P, B], f32, tag="ps_trf", name="ps_trf", bufs=1)
                for fb in range(FB):
                    nc.tensor.transpose(
                        ptf[:, fb * P : (fb + 1) * P],
                        f_nat[fb][:, dc * P : (dc + 1) * P],
                        ident[:],
                    )
                if dc % 2 == 0:
                    nc.scalar.copy(ft[dc][:], ptf[:])
                else:
                    nc.vector.tensor_copy(ft[dc][:], ptf[:])
                ptk = ps1.tile([P, M], f32, tag="ps_trk", name="ps_trk", bufs=1)
                for kb in range(KB):
                    nc.tensor.transpose(
                        ptk[:, kb * P : (kb + 1) * P],
                        kn[kb][:, dc * P : (dc + 1) * P],
                        ident[:],
                    )
                if dc % 2 == 0:
                    nc.vector.tensor_copy(knt[dc][:], ptk[:])
                else:
                    nc.scalar.copy(knt[dc][:], ptk[:])

            # sim = F @ Kn^T per 128-row block, fp32 accumulation in PSUM,
            # then exact top-16 -> clamped, renormalized weights.
            w_sb = [
                persist.tile([P, M], f32, tag=f"w{fb}", name=f"w{fb}")
                for fb in range(FB)
            ]
            for fb in range(FB):
                ps_sim = ps1.tile([P, M], f32, tag="ps_sim", name="ps_sim")
                for dc in range(DC):
                    nc.tensor.matmul(
                        ps_sim[:],
                        ft[dc][:, fb * P : (fb + 1) * P],
                        knt[dc][:],
                        start=(dc == 0),
                        stop=(dc == DC - 1),
                    )
                sim = persist.tile([P, M], f32, tag=f"sim{fb}", name=f"sim{fb}")
                nc.vector.tensor_copy(sim[:], ps_sim[:])

                # two rounds of (top-8, zap-to-0); all top-16 sims are > 0
                # for this distribution so 0 never wins a max.
                t = scratch.tile([P, M], f32, tag="tk_t", name="tk_t")
                m8a = scratch.tile([P, 8], f32, tag="tk_m8a", name="tk_m8a")
                m8b = scratch.tile([P, 8], f32, tag="tk_m8b", name="tk_m8b")
                nc.vector.max(out=m8a[:], in_=sim[:])
                nc.vector.match_replace(
                    out=t[:], in_to_replace=m8a[:], in_values=sim[:], imm_value=0.0
                )
                nc.vector.max(out=m8b[:], in_=t[:])
                nc.vector.match_replace(
                    out=t[:], in_to_replace=m8b[:], in_values=t[:], imm_value=0.0
                )
                # v = sim - t: top-16 keep value, rest -> 0
                v = scratch.tile([P, M], f32, tag="tk_v", name="tk_v")
                nc.vector.tensor_sub(out=v[:], in0=sim[:], in1=t[:])
                # clamp negatives + row-sum in one ACT op
                v2 = scratch.tile([P, M], f32, tag="tk_v2", name="tk_v2")
                rowsum = scratch.tile([P, 1], f32, tag="tk_rs", name="tk_rs")
                nc.scalar.activation(v2[:], v[:], AFT.Relu, accum_out=rowsum[:])
                rs_inv = scratch.tile([P, 1], f32, tag="tk_rsi", name="tk_rsi")
                nc.vector.reciprocal(rs_inv[:], rowsum[:])
                nc.vector.tensor_scalar_mul(w_sb[fb][:], v2[:], rs_inv[:])

            # Transpose W -> WT (fp32r; the DVE copy out of PSUM rounds).
            wt = [
                persist.tile([P, B], f32r, tag=f"wt{kb}", name=f"wt{kb}")
                for kb in range(KB)
            ]
            for kb in range(KB):
                ptw = ps1.tile([P, B], f32, tag="ps_trf", name="ps_trw", bufs=1)
                for fb in range(FB):
                    nc.tensor.transpose(
                        ptw[:, fb * P : (fb + 1) * P],
                        w_sb[fb][:, kb * P : (kb + 1) * P],
                        ident[:],
                    )
                nc.vector.tensor_copy(wt[kb][:], ptw[:])

            # ---- Phase 2: out = W @ mem, fp32r, streamed over columns ----
            for nt in range(N_CHUNKS):
                mem_r = preloaded.pop(nt) if nt in preloaded else load_chunk(nt)
                for fb in range(FB):
                    ot = out_pool.tile([P, NT_CHUNK], f32, tag="ot", name=f"ot{nt}_{fb}")
                    for sub in range(SUBS):
                        ps = psg.tile([P, 512], f32, tag="ps_gemm", name="ps_gemm")
                        for kb in range(KB):
                            nc.tensor.matmul(
                                ps[:],
                                wt[kb][:, fb * P : (fb + 1) * P],
                                mem_r[kb][:, sub * 512 : (sub + 1) * 512],
                                start=(kb == 0),
                                stop=(kb == KB - 1),
                            )
                        dst = ot[:, sub * 512 : (sub + 1) * 512]
                        if (fb + sub) % 2 == 0:
                            nc.vector.tensor_copy(dst, ps[:])
                        else:
                            nc.scalar.copy(dst, ps[:])
                    nc.sync.dma_start(
                        oap[fb * P : (fb + 1) * P,
                            nt * NT_CHUNK : (nt + 1) * NT_CHUNK],
                        ot[:],
                    )

    nc.finalize()
    return nc


def _get_nc():
    global _CACHED_NC
    if _CACHED_NC is None:
        _CACHED_NC = _build_nc()
    return _CACHED_NC


def kernel(features: np.ndarray, keys: np.ndarray, memory: np.ndarray) -> np.ndarray:
    from concourse.bass_utils import run_bass_kernel_spmd

    features = np.ascontiguousarray(np.asarray(features, dtype=np.float32))
    keys = np.ascontiguousarray(np.asarray(keys, dtype=np.float32))
    mem2d = np.asarray(memory, dtype=np.float32).reshape(M, PQ)

    in_maps = []
    for c in range(N_CORES):
        shard = np.ascontiguousarray(mem2d[:, c * NSH : (c + 1) * NSH])
        in_maps.append({"features": features, "keys": keys, "mem": shard})

    nc = _get_nc()
    last_err = None
    for _attempt in range(2):
        try:
            res = run_bass_kernel_spmd(nc, in_maps, core_ids=list(range(N_CORES)))
            break
        except Exception as e:  # transient NRT device errors: retry once
            last_err = e
    else:
        raise last_err

    out = np.concatenate([r["out"] for r in res.results], axis=1)
    return out.reshape(B, 64, 2048)
